# revision 19
# baseline (speedup 1.0000x reference)
"""Multi-head causal attention (B=4, T=2048, 16 heads x 64) on 8 trn2 NeuronCores.

Sharding: tensor-parallel over heads (2 heads/core) for QKV projection +
attention; one AllToAll reshard per batch (head-sharded -> token-sharded),
fired as each batch finishes so only the last one is exposed; the output
projection for each batch's tokens is interleaved one batch behind.

Per-core dataflow (all matmuls in fp32r = full-rate fp32 on the PE):
  - x.T (host-pretransposed) streamed per (batch, 512-token tile); QKV
    projections produce Q.T, K.T [128=2*64 headdim, T] and V.T; V.T is
    PE-transposed into V [ks, d] with an appended ones-column (V_aug).
  - Scores are computed transposed, S.T[ks,tq] = K @ Q.T (K=64 contraction),
    two ks-chunks paired into one [128,1024] PSUM tile so the Exp activation
    runs once per pair; softmax without max-subtraction (scores provably in
    [-0.52, 0.52]); causal masking via skipping fully-masked tiles, one
    128x128 triangular mask multiply per diagonal chunk.
  - A'V and the softmax denominator in one accumulation: lhsT = V_aug
    [ks, 65] (col 64 = ones) -> O.T|denom [65, tq] accumulated over ks.
    The score matmuls for pair k+1 are emitted before the A'V matmuls of
    pair k so the PE never stalls waiting for the Exp.
  - normalize: copy O.T out of PSUM first (frees the accumulator bank),
    then reciprocal of denom row, PE ones-matmul broadcast, DVE mul.
  - output projection over the 8 gathered head-chunks; bias add.

One AllToAll per half-batch (8 total; all but the last hidden under
compute), each moving [8 shards, 128 hd, 128 tok]: for half-batch
(b, jj), shard (j-2jj)*4+q holds tokens b*2048 + j*512 + q*128 + [0,128);
rank r owns (j = 2jj + r//4, q = r%4) of every half-batch = 8 x 128 rows.
"""

import numpy as np

import concourse.bacc as bacc
import concourse.tile as tile
from concourse import mybir
from concourse.bass_utils import run_bass_kernel_spmd

NCORES = 8
B, T, C, H, D = 4, 2048, 1024, 16, 64
TQ = 512          # moving-dim tile for scores / A'V
NKC = T // 128    # ks 128-chunks per batch (16)
NJ = T // TQ      # tq tiles per batch (4)
NCC = C // 128    # contraction chunks for projections (8)
TS = 256          # tokens per A2A shard

f32 = mybir.dt.float32
f32r = mybir.dt.float32r
AF = mybir.ActivationFunctionType


def build_bass():
    nc = bacc.Bacc(None, num_devices=NCORES)

    xT = nc.dram_tensor("xT", [B, C, T], f32, kind="ExternalInput")
    # per-core qkv weights: [3(q,k,v), chunk, row-in-chunk, 2*D]
    w_in = nc.dram_tensor("w", [3, NCC, 128, 2 * D], f32, kind="ExternalInput")
    # proj_w.T chunked: [chunk, row-in-chunk, C]
    pw_in = nc.dram_tensor("pw", [NCC, 128, C], f32, kind="ExternalInput")
    bias_in = nc.dram_tensor("biasb", [128, C], f32, kind="ExternalInput")
    y_out = nc.dram_tensor("y", [B * T // NCORES, C], f32, kind="ExternalOutput")

    ident_np = np.eye(128, dtype=np.float32)
    tri_np = (np.arange(128)[None, :] >= np.arange(128)[:, None]).astype(np.float32)
    ident_dram = nc.inline_tensor(ident_np, name="ident")
    tri_dram = nc.inline_tensor(tri_np, name="trimask")
    onescols_dram = nc.inline_tensor(np.ones((128, NKC), np.float32), name="onescols")
    ones64_dram = nc.inline_tensor(np.ones((1, 64), np.float32), name="ones64")

    with tile.TileContext(nc, num_cores=NCORES) as tc:
        with (
            tc.tile_pool(name="dram", bufs=1, space="DRAM") as dpool,
            tc.tile_pool(name="consts", bufs=1) as consts,
            tc.tile_pool(name="xt", bufs=3) as xt_pool,
            tc.tile_pool(name="qt", bufs=2) as qt_pool,
            tc.tile_pool(name="kt", bufs=2) as kt_pool,
            tc.tile_pool(name="vt", bufs=1) as vt_pool,
            tc.tile_pool(name="vaug", bufs=2) as vaug_pool,
            tc.tile_pool(name="e", bufs=4) as e_pool,
            tc.tile_pool(name="onorm", bufs=4) as onorm_pool,
            tc.tile_pool(name="small", bufs=2) as small_pool,
            tc.tile_pool(name="proj", bufs=2) as proj_pool,
            tc.tile_pool(name="ytile", bufs=2) as y_pool,
            tc.tile_pool(name="ps_a", bufs=2, space="PSUM") as ps_a,
            tc.tile_pool(name="ps_s", bufs=2, space="PSUM") as ps_s,
            tc.tile_pool(name="ps_o", bufs=1, space="PSUM") as ps_o,
        ):
            sends = [
                dpool.tile([NCORES, 2 * D, 128], f32, tag="send", name=f"send{hb}")
                for hb in range(2 * B)
            ]
            recvs = [
                dpool.tile([NCORES, 2 * D, 128], f32, tag="recv", name=f"recv{hb}")
                for hb in range(2 * B)
            ]

            w_sb = consts.tile([128, 3, NCC, 2 * D], f32r)
            nc.sync.dma_start(
                out=w_sb[:], in_=w_in.rearrange("p i r c -> r p i c").bitcast(f32r)
            )
            ident_sb = consts.tile([128, 128], f32)
            nc.sync.dma_start(out=ident_sb[:], in_=ident_dram[:])
            tri_sb = consts.tile([128, 128], f32r)
            nc.sync.dma_start(out=tri_sb[:], in_=tri_dram[:].bitcast(f32r))
            ones64_sb = consts.tile([1, 64], f32r)
            nc.sync.dma_start(out=ones64_sb[:], in_=ones64_dram[:].bitcast(f32r))
            pw_sb = consts.tile([128, NCC, C], f32r)
            bias_sb = consts.tile([128, C], f32)

            def load_proj_consts():
                nc.sync.dma_start(
                    out=pw_sb[:], in_=pw_in.rearrange("i r e -> r i e").bitcast(f32r)
                )
                nc.sync.dma_start(out=bias_sb[:], in_=bias_in[:])

            def proj_units(hb):
                """Output projection units for half-batch hb (y rows hb*128..+128)."""
                oall = proj_pool.tile(
                    [128, NCORES, 128], f32r, tag="oall", name=f"oall{hb}"
                )

                def load():
                    nc.sync.dma_start(
                        out=oall[:],
                        in_=recvs[hb][:].rearrange("i p t -> p i t").bitcast(f32r),
                    )

                def mm(n):
                    pso = ps_a.tile(
                        [128, 512], f32, tag="qkv", name=f"pso{hb}_{n}"
                    )
                    for i in range(NCC):
                        nc.tensor.matmul(
                            pso[:],
                            lhsT=oall[:, i, :],
                            rhs=pw_sb[:, i, n * 512:(n + 1) * 512],
                            start=(i == 0),
                            stop=(i == NCC - 1),
                        )
                    yt = y_pool.tile(
                        [128, 512], f32, tag="yt", name=f"yt{hb}_{n}"
                    )
                    nc.vector.tensor_add(
                        yt[:], pso[:], bias_sb[:, n * 512:(n + 1) * 512]
                    )
                    nc.sync.dma_start(
                        out=y_out[hb * 128:(hb + 1) * 128, n * 512:(n + 1) * 512],
                        in_=yt[:],
                    )

                return [lambda: (load(), mm(0)), lambda: mm(1)]

            def make_qkv(b):
                """Emit xt DMAs eagerly; return (tiles, PE work units) for batch b.

                Each unit is ~1.8us of PE work with no ACT dependency; they are
                interleaved into the previous batch's attention rounds to keep
                the PE dense (and therefore at the warm 2.4 GHz clock) while
                the ACT engine works through the Exp stream."""
                QT = qt_pool.tile([128, T], f32r, tag="QT", name=f"QT{b}")
                KT = kt_pool.tile([128, T], f32r, tag="KT", name=f"KT{b}")
                VT = vt_pool.tile([128, T], f32, tag="VT", name=f"VT{b}")
                VA = vaug_pool.tile([128, NKC, 130], f32r, tag="VA", name=f"VA{b}")
                nc.sync.dma_start(out=VA[:, :, 64], in_=onescols_dram[:].bitcast(f32r))
                nc.sync.dma_start(out=VA[:, :, 129], in_=onescols_dram[:].bitcast(f32r))
                dests = [QT, KT, VT]
                xts = []
                for t4 in range(NJ):
                    xt = xt_pool.tile([128, NCC, TQ], f32r, tag="xt",
                                      name=f"xt{b}_{t4}")
                    nc.sync.dma_start(
                        out=xt[:],
                        in_=xT[b, :, t4 * TQ:(t4 + 1) * TQ]
                        .rearrange("(i p) t -> p i t", p=128)
                        .bitcast(f32r),
                    )
                    xts.append(xt)
                units = []

                def chain(t4, p3):
                    ps = ps_a.tile([128, TQ], f32, tag="qkv",
                                   name=f"qkv{b}_{t4}_{p3}")
                    for i in range(NCC):
                        nc.tensor.matmul(
                            ps[:],
                            lhsT=w_sb[:, p3, i, :],
                            rhs=xts[t4][:, i, :],
                            start=(i == 0),
                            stop=(i == NCC - 1),
                        )
                    nc.scalar.activation(
                        dests[p3][:, t4 * TQ:(t4 + 1) * TQ], ps[:], AF.Copy
                    )

                def transp(kc):
                    pst = ps_a.tile([128, 128], f32, tag="qkv", name=f"pst{b}_{kc}")
                    nc.tensor.transpose(
                        pst[:], VT[:, kc * 128:(kc + 1) * 128], ident_sb[:]
                    )
                    out_ap = VA[:, kc, :].rearrange("p (g s) -> p g s", s=65)[
                        :, :, 0:64
                    ]
                    in_ap = pst[:].rearrange("p (g s) -> p g s", s=64)
                    nc.vector.tensor_copy(out_ap, in_ap)

                for t4 in range(NJ):
                    for p3 in range(3):
                        units.append(lambda t4=t4, p3=p3: chain(t4, p3))
                for kc2 in range(NKC // 2):
                    units.append(
                        lambda kc2=kc2: (transp(2 * kc2), transp(2 * kc2 + 1))
                    )
                return (QT, KT, VA), units

            cur, units = make_qkv(0)
            for u in units:
                u()
            load_proj_consts()

            pending_norm = []
            rnd = 0  # global attention round counter (never reset)

            def fire_a2a(hb):
                nc.gpsimd.collective_compute(
                    "AllToAll",
                    mybir.AluOpType.bypass,
                    replica_groups=[list(range(NCORES))],
                    ins=[sends[hb][:].opt()],
                    outs=[recvs[hb][:].opt()],
                )

            for b in range(B):
                QT, KT, VA = cur
                if b + 1 < B:
                    cur, units = make_qkv(b + 1)
                else:
                    units = []
                # previous batch's projections, once their A2As have landed
                if b >= 1:
                    units = units + proj_units(2 * b - 2) + proj_units(2 * b - 1)
                ui = 0
                rb0 = rnd  # this batch's first round

                # attention for this batch, both heads, chunk-PAIR pipelined
                hb_done = {}
                for j in reversed(range(NJ)):
                    po = [
                        ps_o.tile([65, TQ], f32, tag=f"o{h}", name=f"po{h}_{b}_{j}")
                        for h in (0, 1)
                    ]
                    npairs = 2 * (j + 1)
                    av_queue = []  # exp'd pairs whose A'V is pending

                    def emit_av(item, j=j, po=po, VA=VA):
                        E2, cpair = item
                        for h in (0, 1):
                            for ci in (0, 1):
                                c = 2 * cpair + ci
                                m = c - 4 * j
                                cs = (
                                    slice(ci * TQ, (ci + 1) * TQ)
                                    if m < 0
                                    else slice(ci * TQ + m * 128, (ci + 1) * TQ)
                                )
                                ocs = slice(0, TQ) if m < 0 else slice(m * 128, TQ)
                                nc.tensor.matmul(
                                    po[h][:, ocs],
                                    lhsT=VA[:, c, 65 * h:65 * h + 65],
                                    rhs=E2[h][:, cs],
                                    start=(c == 0),
                                    stop=(c == 4 * j + 3),
                                )

                    for cpair in range(npairs):
                        pss2 = [
                            ps_s.tile([128, 2 * TQ], f32, tag="pss",
                                      name=f"pss{b}_{j}_{cpair}_{h}")
                            for h in (0, 1)
                        ]
                        # h-inner order alternates PE row groups -> the two
                        # heads' K=64 score matmuls run concurrently
                        for ci in (0, 1):
                            c = 2 * cpair + ci
                            for h in (0, 1):
                                nc.tensor.matmul(
                                    pss2[h][:, ci * TQ:(ci + 1) * TQ],
                                    lhsT=KT[
                                        64 * h:64 * (h + 1), c * 128:(c + 1) * 128
                                    ],
                                    rhs=QT[64 * h:64 * (h + 1), j * TQ:(j + 1) * TQ],
                                    start=True,
                                    stop=True,
                                    tile_position=(64 * h, 0),
                                )
                        E2 = []
                        for h in (0, 1):
                            E = e_pool.tile(
                                [128, 2 * TQ], f32r, tag="E",
                                name=f"E{b}_{j}_{cpair}_{h}",
                            )
                            nc.scalar.activation(E[:], pss2[h][:], AF.Exp, scale=0.125)
                            # triangular mask on diagonal chunks
                            for ci in (0, 1):
                                c = 2 * cpair + ci
                                m = c - 4 * j
                                if m >= 0:
                                    sl = slice(
                                        ci * TQ + m * 128, ci * TQ + (m + 1) * 128
                                    )
                                    nc.vector.tensor_mul(E[:, sl], E[:, sl], tri_sb[:])
                            E2.append(E)
                        av_queue.append((E2, cpair))
                        if len(av_queue) > 1:
                            emit_av(av_queue.pop(0))
                        nunit = 1 if (b + 1 < B or rnd - rb0 >= 8) else 0
                        if rnd - rb0 >= 12:
                            nunit += 1
                        for _ in range(nunit):
                            if ui < len(units):
                                units[ui]()
                                ui += 1
                        if pending_norm and rnd >= pending_norm[0][1] + 2:
                            pending_norm.pop(0)[0]()
                        rnd += 1
                    emit_av(av_queue.pop(0))

                    # copy O.T out of PSUM now (frees the accumulator bank);
                    # the rest of the normalize is deferred a round so the
                    # PE never waits on the DVE reciprocal
                    on_raws = []
                    for h in (0, 1):
                        on_raw = onorm_pool.tile(
                            [65, TQ], f32, tag="onr", name=f"onr{b}_{j}_{h}"
                        )
                        nc.vector.tensor_copy(on_raw[:], po[h][:])
                        on_raws.append(on_raw)

                    def norm_rest(b=b, j=j, on_raws=on_raws):
                        for h in (0, 1):
                            on_raw = on_raws[h]
                            rec = small_pool.tile([1, TQ], f32r, tag="rec")
                            with nc.allow_low_precision(reason="f32r == f32 bits"):
                                nc.vector.reciprocal(rec[:], on_raw[64:65, :])
                            den = ps_a.tile(
                                [64, TQ], f32, tag="qkv", name=f"den{b}_{j}_{h}"
                            )
                            nc.tensor.matmul(
                                den[:], lhsT=ones64_sb[:], rhs=rec[:],
                                start=True, stop=True,
                            )
                            on = onorm_pool.tile(
                                [64, TQ], f32, tag="on", name=f"on{b}_{j}_{h}"
                            )
                            nc.vector.tensor_mul(on[:], on_raw[0:64, :], den[:])
                            hb = 2 * b + j // 2
                            for q in range(4):
                                nc.sync.dma_start(
                                    out=sends[hb][
                                        (j % 2) * 4 + q, 64 * h:64 * (h + 1), :
                                    ],
                                    in_=on[:, q * 128:(q + 1) * 128],
                                )
                        hb = 2 * b + j // 2
                        hb_done[hb] = hb_done.get(hb, 0) + 1
                        if hb_done[hb] == 2:
                            fire_a2a(hb)

                    pending_norm.append((norm_rest, rnd))
                for u in units[ui:]:
                    u()
            for fn, _ in pending_norm:
                fn()
            pending_norm = []
            for u in proj_units(2 * B - 2) + proj_units(2 * B - 1):
                u()
    nc.finalize()
    return nc


_NC_CACHE = {}


def _get_nc():
    if "nc" not in _NC_CACHE:
        _NC_CACHE["nc"] = build_bass()
    return _NC_CACHE["nc"]


def _prep_inputs(x, Wk, Wq, Wv, proj_w, proj_b):
    x = np.ascontiguousarray(np.asarray(x, dtype=np.float32))
    xT = np.ascontiguousarray(x.transpose(0, 2, 1))  # [B, C, T]
    pw_r = np.ascontiguousarray(np.asarray(proj_w, np.float32).T).reshape(NCC, 128, C)
    biasb = np.ascontiguousarray(
        np.broadcast_to(np.asarray(proj_b, np.float32), (128, C))
    )
    in_maps = []
    for core in range(NCORES):
        h0 = 2 * core

        def pack(W):
            W2 = np.concatenate(
                [np.asarray(W[h0], np.float32), np.asarray(W[h0 + 1], np.float32)],
                axis=1,
            )  # [C, 2D]
            return W2.reshape(NCC, 128, 2 * D)

        wq = np.stack([pack(Wq), pack(Wk), pack(Wv)], axis=0)  # [3, NCC, 128, 2D]
        in_maps.append(
            {
                "xT": xT,
                "w": np.ascontiguousarray(wq),
                "pw": pw_r,
                "biasb": biasb,
            }
        )
    return in_maps


def _assemble(results):
    """Core r's y rows [hb*128, (hb+1)*128) = tokens
    b*2048 + (2*(hb%2) + r//4)*512 + (r%4)*128 + [0, 128), b = hb//2."""
    out = np.empty((B * T, C), np.float32)
    for r in range(NCORES):
        y = results[r]["y"]
        for hb in range(2 * B):
            b = hb // 2
            base = (2 * (hb % 2) + r // 4) * TQ + (r % 4) * 128
            out[b * T + base:b * T + base + 128] = y[hb * 128:(hb + 1) * 128]
    return out.reshape(B, T, C)


def kernel(x, Wk, Wq, Wv, proj_w, proj_b, _trace=False, _trace_kwargs=None):
    in_maps = _prep_inputs(x, Wk, Wq, Wv, proj_w, proj_b)
    nc = _get_nc()
    kw = {}
    if _trace:
        kw = dict(trace=True, trace_kwargs=_trace_kwargs or {})
    res = run_bass_kernel_spmd(nc, in_maps, core_ids=list(range(NCORES)), **kw)
    out = _assemble(res.results)
    if _trace:
        return out, res
    return out


if __name__ == "__main__":
    d = np.load("/root/problem/cache_io.npz")
    out = kernel(d["x"], d["Wk"], d["Wq"], d["Wv"], d["proj_w"], d["proj_b"])
    ref = d["ref"]
    err = np.abs(out - ref).max() / np.abs(ref).max()
    print("Relative error:", err)
